# revision 8
# baseline (speedup 1.0000x reference)
"""Multi-head attention (B=2, L=2048, D=1024, H=16) on 8 TRN2 NeuronCores.

Sharding: batch (2) x head-group (4 heads each) = 8 shards.
Each core computes q/k/v projections for its 4 heads, attention, and a
partial output projection (its 256 rows of Wo); host sums the 4 partials
per batch and adds bo.

Device dataflow (per core):
  inputs (host-prepped):
    qT  [1024, 2048]  = query[b].T          (d on partitions for matmul)
    mT  [1024, 2048]  = memory[b].T
    wq  [1024, 256]   = Wq[:, J] * 0.125    (scale folded)
    wk, wv [1024, 256]
    bq [1,256]*0.125, bk, bv [1,256]
    wo  [256, 1024]   = Wo[J, :]
    mb  [16, 128]     = memory_bias[b]
  phase 1: qT_h [j, f], kT_h [j, t] (head-dim on partitions), v [t, j]
           (natural), with biases folded in via K=1 ones matmuls and
           v scaled by exp(memory_bias[t]) so softmax bias drops out:
           softmax(s + b) @ v == (exp(s) @ (v * e^b)) / (exp(s) @ e^b)
  phase 2: sT = kT.T@qT -> exp on ScalarE -> AV matmul with an extra
           ones(=e^b) column producing the softmax denominator row for
           free -> normalize -> output projection partial [f, j].
"""

import numpy as np

import concourse.bass as bass
import concourse.tile as tile
from concourse import bacc, mybir
from concourse import bass_utils
from concourse.bass import ts, ds

F32 = mybir.dt.float32
F32R = mybir.dt.float32r

B, LQ, LM, D, H = 2, 2048, 2048, 1024, 16
DH = 64
HPC = 4            # heads per core
JC = HPC * DH      # 256 projection cols per core
NCORE = 8
P = 128
FB = 512           # f-block width
NFB = LQ // FB     # 4
NTC = LM // P      # 16 t-chunks
KD = D // P        # 8 contraction chunks for projections
G = 2              # t-chunks per exp group (psum tile [128, G*512])
NG = NTC // G      # 8 groups

VW = DH + 1        # v columns per head incl. denominator column


def build_kernel(mm_dt=F32R):
    MM = mm_dt
    nc = bacc.Bacc("TRN2", target_bir_lowering=False, debug=False)

    qTd = nc.dram_tensor("qT", [D, LQ], MM, kind="ExternalInput").ap()
    mTd = nc.dram_tensor("mT", [D, LM], MM, kind="ExternalInput").ap()
    wqd = nc.dram_tensor("wq", [D, JC], MM, kind="ExternalInput").ap()
    wkd = nc.dram_tensor("wk", [D, JC], MM, kind="ExternalInput").ap()
    wvd = nc.dram_tensor("wv", [D, JC], MM, kind="ExternalInput").ap()
    bqd = nc.dram_tensor("bq", [1, JC], MM, kind="ExternalInput").ap()
    bkd = nc.dram_tensor("bk", [1, JC], MM, kind="ExternalInput").ap()
    bvd = nc.dram_tensor("bv", [1, JC], MM, kind="ExternalInput").ap()
    wod = nc.dram_tensor("wo", [JC, D], MM, kind="ExternalInput").ap()
    mbd = nc.dram_tensor("mb", [NTC, P], F32, kind="ExternalInput").ap()
    outd = nc.dram_tensor("out", [LQ, D], F32, kind="ExternalOutput").ap()

    with tile.TileContext(nc) as tc:
        with (
            tc.tile_pool(name="persist", bufs=1) as persist,
            tc.tile_pool(name="vpool", bufs=1) as vpool,
            tc.tile_pool(name="consts", bufs=1) as consts,
        ):
            # ---- constants ----
            ones_f = consts.tile([1, FB], F32)
            nc.vector.memset(ones_f[:], 1.0)
            ones_row = consts.tile([1, FB], MM)      # rhs for bias matmuls
            nc.vector.tensor_copy(ones_row[:], ones_f[:])
            ones_col = consts.tile([1, P], MM)       # lhsT for v-bias / R bcast
            nc.vector.tensor_copy(ones_col[:], ones_f[:, 0:P])
            mb_sb = consts.tile([P, NTC], F32)
            nc.sync.dma_start(mb_sb[:], mbd.transpose([1, 0]))
            eb_sb = consts.tile([P, NTC], F32)        # exp(memory_bias), col=tc
            nc.scalar.activation(eb_sb[:], mb_sb[:],
                                 mybir.ActivationFunctionType.Exp)

            # ---- persistent activations ----
            # qT/kT: per head-pair tile [128 (2 heads x 64 dh), L]
            qTp = [persist.tile([P, LQ], MM, name=f"qTp{i}") for i in range(2)]
            kTp = [persist.tile([P, LM], MM, name=f"kTp{i}") for i in range(2)]
            # v: per t-chunk [128 t, 4 heads x (64 v + 1 denom col)]
            v_sb = [vpool.tile([P, HPC * VW], MM, name=f"v{t}")
                    for t in range(NTC)]

            # ================= phase 1: projections =================
            with (
                tc.tile_pool(name="wts", bufs=1) as wts,
                tc.tile_pool(name="rhs", bufs=KD + 2) as rhsp,
                tc.tile_pool(name="pps", bufs=4, space="PSUM") as pps,
                tc.tile_pool(name="ppv", bufs=2, space="PSUM") as ppv,
            ):
                wq_sb = wts.tile([P, KD * JC], MM, name="wq")
                wk_sb = wts.tile([P, KD * JC], MM, name="wk")
                wv_sb = wts.tile([P, KD * JC], MM, name="wv")
                for w_sb, wd in ((wq_sb, wqd), (wk_sb, wkd), (wv_sb, wvd)):
                    for k in range(KD):
                        nc.sync.dma_start(w_sb[:, ts(k, JC)], wd[ds(k * P, P), :])
                bq_sb = wts.tile([1, JC], MM, name="bq")
                bk_sb = wts.tile([1, JC], MM, name="bk")
                bv_sb = wts.tile([1, JC], MM, name="bv")
                nc.sync.dma_start(bq_sb[:], bqd[:])
                nc.sync.dma_start(bk_sb[:], bkd[:])
                nc.sync.dma_start(bv_sb[:], bvd[:])

                # --- q projection: qTp[hp][j, f] ---
                for fb in range(NFB):
                    chunks = []
                    for k in range(KD):
                        c = rhsp.tile([P, FB], MM, name="qchunk")
                        nc.sync.dma_start(c[:], qTd[ds(k * P, P), ts(fb, FB)])
                        chunks.append(c)
                    for hp in range(2):
                        ps = pps.tile([P, FB], F32)
                        for k in range(KD):
                            nc.tensor.matmul(
                                ps[:], wq_sb[:, ds(k * JC + hp * P, P)],
                                chunks[k][:],
                                start=(k == 0), stop=False)
                        nc.tensor.matmul(
                            ps[:], bq_sb[:, ds(hp * P, P)], ones_row[:],
                            start=False, stop=True)
                        nc.vector.tensor_copy(qTp[hp][:, ts(fb, FB)], ps[:])

                # --- k and v projections, sharing mT chunk loads ---
                for tw in range(NFB):   # t-window of 512
                    chunks = []
                    for k in range(KD):
                        c = rhsp.tile([P, FB], MM, name="mchunk")
                        nc.sync.dma_start(c[:], mTd[ds(k * P, P), ts(tw, FB)])
                        chunks.append(c)
                    for hp in range(2):
                        ps = pps.tile([P, FB], F32)
                        for k in range(KD):
                            nc.tensor.matmul(
                                ps[:], wk_sb[:, ds(k * JC + hp * P, P)],
                                chunks[k][:],
                                start=(k == 0), stop=False)
                        nc.tensor.matmul(
                            ps[:], bk_sb[:, ds(hp * P, P)], ones_row[:],
                            start=False, stop=True)
                        nc.vector.tensor_copy(kTp[hp][:, ts(tw, FB)], ps[:])
                    for s in range(4):
                        t = tw * 4 + s
                        psv = ppv.tile([P, JC], F32)
                        for k in range(KD):
                            nc.tensor.matmul(
                                psv[:], chunks[k][:, ds(s * P, P)],
                                wv_sb[:, ts(k, JC)],
                                start=(k == 0), stop=False)
                        nc.tensor.matmul(
                            psv[:], ones_col[:], bv_sb[:],
                            start=False, stop=True)
                        # evict: v * exp(mb[t]) into strided [t, 4x65] layout
                        dst = v_sb[t].rearrange("p (h c) -> p h c", h=HPC)
                        nc.vector.tensor_scalar_mul(
                            dst[:, :, 0:DH],
                            psv[:].rearrange("p (h c) -> p h c", h=HPC),
                            eb_sb[:, ds(t, 1)])
                        for h in range(HPC):
                            nc.vector.tensor_copy(
                                dst[:, ds(h, 1), ds(DH, 1)], eb_sb[:, ds(t, 1)])

            # ================= phase 2: attention + out-proj =================
            wo_sb = [persist.tile([DH, D], MM, name=f"wo{i}") for i in range(HPC)]
            for i in range(HPC):
                nc.sync.dma_start(wo_sb[i][:], wod[ds(i * DH, DH), :])

            with (
                tc.tile_pool(name="expp", bufs=18) as expp,
                tc.tile_pool(name="attnp", bufs=8) as attnp,
                tc.tile_pool(name="rp", bufs=4) as rp,
                tc.tile_pool(name="rbp", bufs=4) as rbp,
                tc.tile_pool(name="osb", bufs=4) as osb,
                tc.tile_pool(name="psl", bufs=2, space="PSUM") as psl,
                tc.tile_pool(name="psav", bufs=2, space="PSUM") as psav,
                tc.tile_pool(name="psr", bufs=1, space="PSUM") as psr,
                tc.tile_pool(name="pso", bufs=1, space="PSUM") as pso,
            ):
                for fb in range(NFB):
                    attn = []   # [64, FB] per head, this f-block
                    for hp in range(2):
                        # logits + exp, interleaving the two heads of the
                        # pair (row strips 0-63 / 64-127 can overlap in PE)
                        exps = [[], []]
                        for g in range(NG):
                            pls = [psl.tile([P, G * FB], F32, name="pls") for _ in range(2)]
                            for s in range(G):
                                t = g * G + s
                                for h2 in range(2):
                                    nc.tensor.matmul(
                                        pls[h2][:, ts(s, FB)],
                                        kTp[hp][ds(h2 * DH, DH), ts(t, P)],
                                        qTp[hp][ds(h2 * DH, DH), ts(fb, FB)],
                                        start=True, stop=True)
                            for h2 in range(2):
                                e = expp.tile([P, G * FB], MM, name="exps")
                                nc.scalar.activation(
                                    e[:], pls[h2][:],
                                    mybir.ActivationFunctionType.Exp)
                                exps[h2].append(e)
                        for h2 in range(2):
                            h = 2 * hp + h2
                            av = psav.tile([VW, FB], F32)
                            for g in range(NG):
                                for s in range(G):
                                    t = g * G + s
                                    nc.tensor.matmul(
                                        av[:], v_sb[t][:, ds(h * VW, VW)],
                                        exps[h2][g][:, ts(s, FB)],
                                        start=(t == 0), stop=(t == NTC - 1))
                            # denominator -> reciprocal -> broadcast to rows
                            rrow = rp.tile([1, FB], MM, name="rrow")
                            with nc.allow_low_precision(
                                    reason="f32r reciprocal feeds PE bcast"):
                                nc.vector.reciprocal(rrow[:], av[ds(DH, 1), :])
                            rb_ps = psr.tile([P, FB], F32)
                            nc.tensor.matmul(rb_ps[:], ones_col[:],
                                             rrow[:], start=True, stop=True)
                            rb = rbp.tile([DH, FB], F32, name="rb")
                            nc.vector.tensor_copy(rb[:], rb_ps[0:DH, :])
                            a = attnp.tile([DH, FB], MM, name="attn")
                            nc.vector.tensor_tensor(
                                a[:], av[0:DH, :], rb[:],
                                op=mybir.AluOpType.mult)
                            attn.append(a)
                    # ---- output projection for this f-block ----
                    for fc in range(4):
                        for jb in range(2):
                            ops = pso.tile([P, FB], F32)
                            for h in range(HPC):
                                nc.tensor.matmul(
                                    ops[:],
                                    attn[h][:, ds(fc * P, P)],
                                    wo_sb[h][:, ts(jb, FB)],
                                    start=(h == 0), stop=(h == HPC - 1))
                            o = osb.tile([P, FB], F32, name="osb")
                            nc.vector.tensor_copy(o[:], ops[:])
                            nc.sync.dma_start(
                                outd[ds(fb * FB + fc * P, P), ts(jb, FB)],
                                o[:])

    nc.compile()
    return nc


_CACHE = {}


def _get_module():
    if "nc" not in _CACHE:
        _CACHE["nc"] = build_kernel()
    return _CACHE["nc"]


def make_in_maps(query, memory, memory_bias, Wq, bq, Wk, bk, Wv, bv, Wo, bo):
    query = np.asarray(query, np.float32)
    memory = np.asarray(memory, np.float32)
    memory_bias = np.asarray(memory_bias, np.float32)
    Wq = np.asarray(Wq, np.float32)
    bq = np.asarray(bq, np.float32)
    Wk = np.asarray(Wk, np.float32)
    bk = np.asarray(bk, np.float32)
    Wv = np.asarray(Wv, np.float32)
    bv = np.asarray(bv, np.float32)
    Wo = np.asarray(Wo, np.float32)
    s = np.float32(DH ** -0.5)

    qT = [np.ascontiguousarray(query[b].T) for b in range(B)]
    mT = [np.ascontiguousarray(memory[b].T) for b in range(B)]
    in_maps = []
    for c in range(NCORE):
        b, g = divmod(c, B * 2)
        J = slice(g * JC, (g + 1) * JC)
        in_maps.append({
            "qT": qT[b],
            "mT": mT[b],
            "wq": np.ascontiguousarray(Wq[:, J]) * s,
            "wk": np.ascontiguousarray(Wk[:, J]),
            "wv": np.ascontiguousarray(Wv[:, J]),
            "bq": (bq[J] * s).reshape(1, JC),
            "bk": bk[J].reshape(1, JC).copy(),
            "bv": bv[J].reshape(1, JC).copy(),
            "wo": np.ascontiguousarray(Wo[J, :]),
            "mb": memory_bias[b].reshape(NTC, P).copy(),
        })
    return in_maps


def gather_output(results, bo):
    bo = np.asarray(bo, np.float32)
    out = np.empty((B, LQ, D), np.float32)
    for b in range(B):
        acc = results[4 * b]["out"].astype(np.float32)
        for g in range(1, 4):
            acc = acc + results[4 * b + g]["out"]
        out[b] = acc + bo
    return out


def kernel(**inputs):
    nc = _get_module()
    in_maps = make_in_maps(**inputs)
    res = bass_utils.run_bass_kernel_spmd(nc, in_maps,
                                          core_ids=list(range(NCORE)))
    return gather_output(res.results, inputs["bo"])


# revision 15
# speedup vs baseline: 1.2121x; 1.2121x over previous
"""Multi-head attention (B=2, L=2048, D=1024, H=16) on 8 TRN2 NeuronCores.

Sharding: batch (2) x head-group (4 heads each) = 8 shards.
Each core computes q/k/v projections for its 4 heads, attention, and a
partial output projection (its 256 rows of Wo); host sums the 4 partials
per batch and adds bo.

Device dataflow (per core):
  inputs (host-prepped):
    qT  [1024, 2048]  = query[b].T          (d on partitions for matmul)
    mT  [1024, 2048]  = memory[b].T
    wq  [1024, 256]   = Wq[:, J] * 0.125    (scale folded)
    wk, wv [1024, 256]
    bq [1,256]*0.125, bk, bv [1,256]
    wo  [256, 1024]   = Wo[J, :]
    mb  [16, 128]     = memory_bias[b]
  phase 1: qT_h [j, f], kT_h [j, t] (head-dim on partitions), v [t, j]
           (natural), with biases folded in via K=1 ones matmuls and
           v scaled by exp(memory_bias[t]) so softmax bias drops out:
           softmax(s + b) @ v == (exp(s) @ (v * e^b)) / (exp(s) @ e^b)
  phase 2: sT = kT.T@qT -> exp on ScalarE -> AV matmul with an extra
           ones(=e^b) column producing the softmax denominator row for
           free -> normalize -> output projection partial [f, j].
"""

import numpy as np

import concourse.bass as bass
import concourse.tile as tile
from concourse import bacc, mybir
from concourse import bass_utils
from concourse.bass import ts, ds

F32 = mybir.dt.float32
F32R = mybir.dt.float32r
BF16 = mybir.dt.bfloat16

B, LQ, LM, D, H = 2, 2048, 2048, 1024, 16
DH = 64
HPC = 4            # heads per core
JC = HPC * DH      # 256 projection cols per core
NCORE = 8
P = 128
FB = 512           # f-block width
NFB = LQ // FB     # 4
NTC = LM // P      # 16 t-chunks
KD = D // P        # 8 contraction chunks for projections
G = 2              # t-chunks per exp group (psum tile [128, G*512])
NG = NTC // G      # 8 groups

VW = DH + 1        # v columns per head incl. denominator column
VS = DH + 2        # v column stride per head (padded: keep 4B alignment in bf16)


def build_kernel(mm_dt=BF16, debug_taps=False):
    MM = mm_dt
    nc = bacc.Bacc("TRN2", target_bir_lowering=False, debug=False)

    qTd = nc.dram_tensor("qT", [D, LQ], MM, kind="ExternalInput").ap()
    mTd = nc.dram_tensor("mT", [D, LM], MM, kind="ExternalInput").ap()
    wqd = nc.dram_tensor("wq", [D, JC], MM, kind="ExternalInput").ap()
    wkd = nc.dram_tensor("wk", [D, JC], MM, kind="ExternalInput").ap()
    wvd = nc.dram_tensor("wv", [D, JC], MM, kind="ExternalInput").ap()
    bqd = nc.dram_tensor("bq", [1, JC], MM, kind="ExternalInput").ap()
    bkd = nc.dram_tensor("bk", [1, JC], MM, kind="ExternalInput").ap()
    bvd = nc.dram_tensor("bv", [1, JC], MM, kind="ExternalInput").ap()
    wod = nc.dram_tensor("wo", [JC, D], MM, kind="ExternalInput").ap()
    mbd = nc.dram_tensor("mb", [NTC, P], F32, kind="ExternalInput").ap()
    outd = nc.dram_tensor("out", [LQ, D], F32, kind="ExternalOutput").ap()
    taps = {}
    if debug_taps:
        for nm, shp in (("t_qTp0", [P, LQ]), ("t_kTp0", [P, LM]),
                        ("t_v0", [P, HPC * VS]), ("t_exp0", [P, G * FB]),
                        ("t_attn0", [DH, FB]), ("t_rb0", [DH, FB]),
                        ("t_av0", [VW, FB])):
            taps[nm] = nc.dram_tensor(nm, shp, mybir.dt.bfloat16,
                                      kind="ExternalOutput").ap()
        taps["t_rf0"] = nc.dram_tensor("t_rf0", [1, FB], F32,
                                       kind="ExternalOutput").ap()

    with tile.TileContext(nc) as tc:
        with (
            tc.tile_pool(name="persist", bufs=1) as persist,
            tc.tile_pool(name="vpool", bufs=1) as vpool,
            tc.tile_pool(name="consts", bufs=1) as consts,
        ):
            # ---- constants ----
            ones_f = consts.tile([1, FB], F32)
            nc.vector.memset(ones_f[:], 1.0)
            ones_row = consts.tile([1, FB], MM)      # rhs for bias matmuls
            nc.vector.tensor_copy(ones_row[:], ones_f[:])
            ones_col = consts.tile([1, P], MM)       # lhsT for v-bias / R bcast
            nc.vector.tensor_copy(ones_col[:], ones_f[:, 0:P])
            mb_sb = consts.tile([P, NTC], F32)
            nc.sync.dma_start(mb_sb[:], mbd.transpose([1, 0]))
            eb_sb = consts.tile([P, NTC], F32)        # exp(memory_bias), col=tc
            nc.scalar.activation(eb_sb[:], mb_sb[:],
                                 mybir.ActivationFunctionType.Exp)

            # ---- persistent activations ----
            # qT/kT: per head-pair tile [128 (2 heads x 64 dh), L]
            qTp = [persist.tile([P, LQ], MM, name=f"qTp{i}") for i in range(2)]
            kTp = [persist.tile([P, LM], MM, name=f"kTp{i}") for i in range(2)]
            # v: per t-chunk [128 t, 4 heads x (64 v + 1 denom col)]
            v_sb = [vpool.tile([P, HPC * VS], MM, name=f"v{t}")
                    for t in range(NTC)]

            # ================= phase 1: projections =================
            with (
                tc.tile_pool(name="wts", bufs=1) as wts,
                tc.tile_pool(name="rhs", bufs=KD + 2) as rhsp,
                tc.tile_pool(name="pps", bufs=4, space="PSUM") as pps,
                tc.tile_pool(name="ppv", bufs=2, space="PSUM") as ppv,
            ):
                wq_sb = wts.tile([P, KD * JC], MM, name="wq")
                wk_sb = wts.tile([P, KD * JC], MM, name="wk")
                wv_sb = wts.tile([P, KD * JC], MM, name="wv")
                for w_sb, wd in ((wq_sb, wqd), (wk_sb, wkd), (wv_sb, wvd)):
                    for k in range(KD):
                        nc.sync.dma_start(w_sb[:, ts(k, JC)], wd[ds(k * P, P), :])
                bq_sb = wts.tile([1, JC], MM, name="bq")
                bk_sb = wts.tile([1, JC], MM, name="bk")
                bv_sb = wts.tile([1, JC], MM, name="bv")
                nc.sync.dma_start(bq_sb[:], bqd[:])
                nc.sync.dma_start(bk_sb[:], bkd[:])
                nc.sync.dma_start(bv_sb[:], bvd[:])

                # --- q projection: qTp[hp][j, f] ---
                for fb in range(NFB):
                    chunks = []
                    for k in range(KD):
                        c = rhsp.tile([P, FB], MM, name="qchunk")
                        nc.sync.dma_start(c[:], qTd[ds(k * P, P), ts(fb, FB)])
                        chunks.append(c)
                    for hp in range(2):
                        ps = pps.tile([P, FB], F32)
                        for k in range(KD):
                            nc.tensor.matmul(
                                ps[:], wq_sb[:, ds(k * JC + hp * P, P)],
                                chunks[k][:],
                                start=(k == 0), stop=False)
                        nc.tensor.matmul(
                            ps[:], bq_sb[:, ds(hp * P, P)], ones_row[:],
                            start=False, stop=True)
                        nc.vector.tensor_copy(qTp[hp][:, ts(fb, FB)], ps[:])

                # --- k and v projections, sharing mT chunk loads ---
                for tw in range(NFB):   # t-window of 512
                    chunks = []
                    for k in range(KD):
                        c = rhsp.tile([P, FB], MM, name="mchunk")
                        nc.sync.dma_start(c[:], mTd[ds(k * P, P), ts(tw, FB)])
                        chunks.append(c)
                    for hp in range(2):
                        ps = pps.tile([P, FB], F32)
                        for k in range(KD):
                            nc.tensor.matmul(
                                ps[:], wk_sb[:, ds(k * JC + hp * P, P)],
                                chunks[k][:],
                                start=(k == 0), stop=False)
                        nc.tensor.matmul(
                            ps[:], bk_sb[:, ds(hp * P, P)], ones_row[:],
                            start=False, stop=True)
                        nc.vector.tensor_copy(kTp[hp][:, ts(tw, FB)], ps[:])
                    for s in range(4):
                        t = tw * 4 + s
                        psv = ppv.tile([P, JC], F32)
                        for k in range(KD):
                            nc.tensor.matmul(
                                psv[:], chunks[k][:, ds(s * P, P)],
                                wv_sb[:, ts(k, JC)],
                                start=(k == 0), stop=False)
                        nc.tensor.matmul(
                            psv[:], ones_col[:], bv_sb[:],
                            start=False, stop=True)
                        # evict: v * exp(mb[t]) into strided [t, 4x65] layout
                        dst = v_sb[t].rearrange("p (h c) -> p h c", h=HPC)
                        nc.vector.tensor_scalar_mul(
                            dst[:, :, 0:DH],
                            psv[:].rearrange("p (h c) -> p h c", h=HPC),
                            eb_sb[:, ds(t, 1)])
                        for h in range(HPC):
                            nc.vector.tensor_copy(
                                dst[:, ds(h, 1), ds(DH, 1)], eb_sb[:, ds(t, 1)])

            # ================= phase 2: attention + out-proj =================
            if debug_taps:
                nc.sync.dma_start(taps["t_qTp0"], qTp[0][:])
                nc.sync.dma_start(taps["t_kTp0"], kTp[0][:])
                nc.sync.dma_start(taps["t_v0"], v_sb[0][:])
            wo_sb = [persist.tile([DH, D], MM, name=f"wo{i}") for i in range(HPC)]
            for i in range(HPC):
                nc.sync.dma_start(wo_sb[i][:], wod[ds(i * DH, DH), :])

            with (
                tc.tile_pool(name="expp", bufs=18) as expp,
                tc.tile_pool(name="attnp", bufs=8) as attnp,
                tc.tile_pool(name="rp", bufs=4) as rp,
                tc.tile_pool(name="rbp", bufs=4) as rbp,
                tc.tile_pool(name="osb", bufs=4) as osb,
                tc.tile_pool(name="psl", bufs=2, space="PSUM") as psl,
                tc.tile_pool(name="psav", bufs=2, space="PSUM") as psav,
                tc.tile_pool(name="psr", bufs=1, space="PSUM") as psr,
                tc.tile_pool(name="pso", bufs=1, space="PSUM") as pso,
            ):
                for fb in range(NFB):
                    attn = []   # [64, FB] per head, this f-block
                    for hp in range(2):
                        # logits + exp, interleaving the two heads of the
                        # pair (row strips 0-63 / 64-127 can overlap in PE)
                        exps = [[], []]
                        for g in range(NG):
                            pls = [psl.tile([P, G * FB], F32, name="pls") for _ in range(2)]
                            for s in range(G):
                                t = g * G + s
                                for h2 in range(2):
                                    nc.tensor.matmul(
                                        pls[h2][:, ts(s, FB)],
                                        kTp[hp][ds(h2 * DH, DH), ts(t, P)],
                                        qTp[hp][ds(h2 * DH, DH), ts(fb, FB)],
                                        start=True, stop=True)
                            for h2 in range(2):
                                e = expp.tile([P, G * FB], MM, name="exps")
                                nc.scalar.activation(
                                    e[:], pls[h2][:],
                                    mybir.ActivationFunctionType.Exp)
                                exps[h2].append(e)
                                if debug_taps and fb == 0 and hp == 0 \
                                        and g == 0 and h2 == 0:
                                    nc.sync.dma_start(taps["t_exp0"], e[:])
                        for h2 in range(2):
                            h = 2 * hp + h2
                            av = psav.tile([VW, FB], F32)
                            for g in range(NG):
                                for s in range(G):
                                    t = g * G + s
                                    nc.tensor.matmul(
                                        av[:], v_sb[t][:, ds(h * VS, VW)],
                                        exps[h2][g][:, ts(s, FB)],
                                        start=(t == 0), stop=(t == NTC - 1))
                            # denominator -> reciprocal -> broadcast to rows
                            if debug_taps and fb == 0 and hp == 0 and h2 == 0:
                                avt = rbp.tile([VW, FB], MM, name="avt")
                                nc.vector.tensor_copy(avt[:], av[:])
                                nc.sync.dma_start(taps["t_av0"], avt[:])
                            # custom-DVE ops ignore base_partition on PSUM
                            # inputs: stage the denominator row to SBUF first
                            dn = rp.tile([1, FB], F32, name="dn")
                            nc.vector.tensor_copy(dn[:], av[ds(DH, 1), :])
                            rf = rp.tile([1, FB], F32, name="rf")
                            nc.vector.reciprocal_approx_fast(rf[:], dn[:])
                            rrow = rp.tile([1, FB], MM, name="rrow")
                            nc.vector.tensor_copy(rrow[:], rf[:])
                            if debug_taps and fb == 0 and hp == 0 and h2 == 0:
                                nc.sync.dma_start(taps["t_rf0"], rf[:])
                            rb_ps = psr.tile([P, FB], F32)
                            nc.tensor.matmul(rb_ps[:], ones_col[:],
                                             rrow[:], start=True, stop=True)
                            rb = rbp.tile([DH, FB], F32, name="rb")
                            nc.vector.tensor_copy(rb[:], rb_ps[0:DH, :])
                            a = attnp.tile([DH, FB], MM, name="attn")
                            nc.vector.tensor_tensor(
                                a[:], av[0:DH, :], rb[:],
                                op=mybir.AluOpType.mult)
                            if debug_taps and fb == 0 and h == 0:
                                nc.sync.dma_start(taps["t_attn0"], a[:])
                                rbt = rbp.tile([DH, FB], MM, name="rbt")
                                nc.vector.tensor_copy(rbt[:], rb[:])
                                nc.sync.dma_start(taps["t_rb0"], rbt[:])
                            attn.append(a)
                    # ---- output projection for this f-block ----
                    for fc in range(4):
                        for jb in range(2):
                            ops = pso.tile([P, FB], F32)
                            for h in range(HPC):
                                nc.tensor.matmul(
                                    ops[:],
                                    attn[h][:, ds(fc * P, P)],
                                    wo_sb[h][:, ts(jb, FB)],
                                    start=(h == 0), stop=(h == HPC - 1))
                            o = osb.tile([P, FB], F32, name="osb")
                            nc.vector.tensor_copy(o[:], ops[:])
                            nc.sync.dma_start(
                                outd[ds(fb * FB + fc * P, P), ts(jb, FB)],
                                o[:])

    nc.compile()
    return nc


_CACHE = {}


def _get_module():
    if "nc" not in _CACHE:
        _CACHE["nc"] = build_kernel()
    return _CACHE["nc"]


def make_in_maps(query, memory, memory_bias, Wq, bq, Wk, bk, Wv, bv, Wo, bo,
                 mm_np=None):
    if mm_np is None:
        import ml_dtypes
        mm_np = ml_dtypes.bfloat16
    query = np.asarray(query, np.float32)
    memory = np.asarray(memory, np.float32)
    memory_bias = np.asarray(memory_bias, np.float32)
    Wq = np.asarray(Wq, np.float32)
    bq = np.asarray(bq, np.float32)
    Wk = np.asarray(Wk, np.float32)
    bk = np.asarray(bk, np.float32)
    Wv = np.asarray(Wv, np.float32)
    bv = np.asarray(bv, np.float32)
    Wo = np.asarray(Wo, np.float32)
    s = np.float32(DH ** -0.5)

    qT = [np.ascontiguousarray(query[b].T).astype(mm_np) for b in range(B)]
    mT = [np.ascontiguousarray(memory[b].T).astype(mm_np) for b in range(B)]
    in_maps = []
    for c in range(NCORE):
        b, g = divmod(c, B * 2)
        J = slice(g * JC, (g + 1) * JC)
        in_maps.append({
            "qT": qT[b],
            "mT": mT[b],
            "wq": (np.ascontiguousarray(Wq[:, J]) * s).astype(mm_np),
            "wk": np.ascontiguousarray(Wk[:, J]).astype(mm_np),
            "wv": np.ascontiguousarray(Wv[:, J]).astype(mm_np),
            "bq": (bq[J] * s).reshape(1, JC).astype(mm_np),
            "bk": bk[J].reshape(1, JC).astype(mm_np),
            "bv": bv[J].reshape(1, JC).astype(mm_np),
            "wo": np.ascontiguousarray(Wo[J, :]).astype(mm_np),
            "mb": memory_bias[b].reshape(NTC, P).copy(),
        })
    return in_maps


def gather_output(results, bo):
    bo = np.asarray(bo, np.float32)
    out = np.empty((B, LQ, D), np.float32)
    for b in range(B):
        acc = results[4 * b]["out"].astype(np.float32)
        for g in range(1, 4):
            acc = acc + results[4 * b + g]["out"]
        out[b] = acc + bo
    return out


def kernel(**inputs):
    nc = _get_module()
    in_maps = make_in_maps(**inputs)
    res = bass_utils.run_bass_kernel_spmd(nc, in_maps,
                                          core_ids=list(range(NCORE)))
    return gather_output(res.results, inputs["bo"])


# revision 17
# speedup vs baseline: 1.4711x; 1.2137x over previous
"""Multi-head attention (B=2, L=2048, D=1024, H=16) on 8 TRN2 NeuronCores.

Sharding: batch (2) x head-group (4 heads each) = 8 shards.
Each core computes q/k/v projections for its 4 heads, attention, and a
partial output projection (its 256 rows of Wo); host sums the 4 partials
per batch and adds bo.

Device dataflow (per core):
  inputs (host-prepped):
    qT  [1024, 2048]  = query[b].T          (d on partitions for matmul)
    mT  [1024, 2048]  = memory[b].T
    wq  [1024, 256]   = Wq[:, J] * 0.125    (scale folded)
    wk, wv [1024, 256]
    bq [1,256]*0.125, bk, bv [1,256]
    wo  [256, 1024]   = Wo[J, :]
    mb  [16, 128]     = memory_bias[b]
  phase 1: qT_h [j, f], kT_h [j, t] (head-dim on partitions), v [t, j]
           (natural), with biases folded in via K=1 ones matmuls and
           v scaled by exp(memory_bias[t]) so softmax bias drops out:
           softmax(s + b) @ v == (exp(s) @ (v * e^b)) / (exp(s) @ e^b)
  phase 2: sT = kT.T@qT -> exp on ScalarE -> AV matmul with an extra
           ones(=e^b) column producing the softmax denominator row for
           free -> normalize -> output projection partial [f, j].
"""

import numpy as np

import concourse.bass as bass
import concourse.tile as tile
from concourse import bacc, mybir
from concourse import bass_utils
from concourse.bass import ts, ds

F32 = mybir.dt.float32
F32R = mybir.dt.float32r
BF16 = mybir.dt.bfloat16

B, LQ, LM, D, H = 2, 2048, 2048, 1024, 16
DH = 64
HPC = 4            # heads per core
JC = HPC * DH      # 256 projection cols per core
NCORE = 8
P = 128
FB = 512           # f-block width
NFB = LQ // FB     # 4
NTC = LM // P      # 16 t-chunks
KD = D // P        # 8 contraction chunks for projections
G = 2              # t-chunks per exp group (psum tile [128, G*512])
NG = NTC // G      # 8 groups

VW = DH + 1        # v columns per head incl. denominator column
VS = DH + 2        # v column stride per head (padded: keep 4B alignment in bf16)


def build_kernel(mm_dt=BF16, debug_taps=False):
    MM = mm_dt
    nc = bacc.Bacc("TRN2", target_bir_lowering=False, debug=False)

    qTd = nc.dram_tensor("qT", [D, LQ], MM, kind="ExternalInput").ap()
    mTd = nc.dram_tensor("mT", [D, LM], MM, kind="ExternalInput").ap()
    wqd = nc.dram_tensor("wq", [D, JC], MM, kind="ExternalInput").ap()
    wkd = nc.dram_tensor("wk", [D, JC], MM, kind="ExternalInput").ap()
    wvd = nc.dram_tensor("wv", [D, JC], MM, kind="ExternalInput").ap()
    bqd = nc.dram_tensor("bq", [1, JC], MM, kind="ExternalInput").ap()
    bkd = nc.dram_tensor("bk", [1, JC], MM, kind="ExternalInput").ap()
    bvd = nc.dram_tensor("bv", [1, JC], MM, kind="ExternalInput").ap()
    wod = nc.dram_tensor("wo", [JC, D], MM, kind="ExternalInput").ap()
    mbd = nc.dram_tensor("mb", [NTC, P], F32, kind="ExternalInput").ap()
    outd = nc.dram_tensor("out", [LQ, D], F32, kind="ExternalOutput").ap()
    taps = {}
    if debug_taps:
        for nm, shp in (("t_qTp0", [P, LQ]), ("t_kTp0", [P, LM]),
                        ("t_v0", [P, HPC * VS]), ("t_exp0", [P, G * FB]),
                        ("t_attn0", [DH, FB]), ("t_rb0", [DH, FB]),
                        ("t_av0", [VW, FB])):
            taps[nm] = nc.dram_tensor(nm, shp, mybir.dt.bfloat16,
                                      kind="ExternalOutput").ap()
        taps["t_rf0"] = nc.dram_tensor("t_rf0", [1, FB], F32,
                                       kind="ExternalOutput").ap()

    with tile.TileContext(nc) as tc:
        with (
            tc.tile_pool(name="persist", bufs=1) as persist,
            tc.tile_pool(name="vpool", bufs=1) as vpool,
            tc.tile_pool(name="consts", bufs=1) as consts,
        ):
            # ---- constants ----
            ones_f = consts.tile([1, FB], F32)
            nc.vector.memset(ones_f[:], 1.0)
            ones_row = consts.tile([1, FB], MM)      # rhs for bias matmuls
            nc.vector.tensor_copy(ones_row[:], ones_f[:])
            ones_col = consts.tile([1, P], MM)       # lhsT for v-bias / R bcast
            nc.vector.tensor_copy(ones_col[:], ones_f[:, 0:P])
            mb_sb = consts.tile([P, NTC], F32)
            nc.sync.dma_start(mb_sb[:], mbd.transpose([1, 0]))
            eb_sb = consts.tile([P, NTC], F32)        # exp(memory_bias), col=tc
            nc.scalar.activation(eb_sb[:], mb_sb[:],
                                 mybir.ActivationFunctionType.Exp)

            # ---- persistent activations ----
            # qT/kT: per head-pair tile [128 (2 heads x 64 dh), L]
            qTp = [persist.tile([P, LQ], MM, name=f"qTp{i}") for i in range(2)]
            kTp = [persist.tile([P, LM], MM, name=f"kTp{i}") for i in range(2)]
            # v: per t-chunk [128 t, 4 heads x (64 v + 1 denom col)]
            v_sb = [vpool.tile([P, HPC * VS], MM, name=f"v{t}")
                    for t in range(NTC)]

            # ================= phase 1: projections =================
            with (
                tc.tile_pool(name="wts", bufs=1) as wts,
                tc.tile_pool(name="rhs", bufs=KD + 2) as rhsp,
                tc.tile_pool(name="pps", bufs=4, space="PSUM") as pps,
                tc.tile_pool(name="ppv", bufs=2, space="PSUM") as ppv,
            ):
                wq_sb = wts.tile([P, KD * JC], MM, name="wq")
                wk_sb = wts.tile([P, KD * JC], MM, name="wk")
                wv_sb = wts.tile([P, KD * JC], MM, name="wv")
                for w_sb, wd in ((wq_sb, wqd), (wk_sb, wkd), (wv_sb, wvd)):
                    for k in range(KD):
                        nc.sync.dma_start(w_sb[:, ts(k, JC)], wd[ds(k * P, P), :])
                bq_sb = wts.tile([1, JC], MM, name="bq")
                bk_sb = wts.tile([1, JC], MM, name="bk")
                bv_sb = wts.tile([1, JC], MM, name="bv")
                nc.sync.dma_start(bq_sb[:], bqd[:])
                nc.sync.dma_start(bk_sb[:], bkd[:])
                nc.sync.dma_start(bv_sb[:], bvd[:])

                # --- q projection: qTp[hp][j, f] ---
                for fb in range(NFB):
                    chunks = []
                    for k in range(KD):
                        c = rhsp.tile([P, FB], MM, name="qchunk")
                        nc.sync.dma_start(c[:], qTd[ds(k * P, P), ts(fb, FB)])
                        chunks.append(c)
                    for hp in range(2):
                        ps = pps.tile([P, FB], F32)
                        for k in range(KD):
                            nc.tensor.matmul(
                                ps[:], wq_sb[:, ds(k * JC + hp * P, P)],
                                chunks[k][:],
                                start=(k == 0), stop=False)
                        nc.tensor.matmul(
                            ps[:], bq_sb[:, ds(hp * P, P)], ones_row[:],
                            start=False, stop=True)
                        nc.vector.tensor_copy(qTp[hp][:, ts(fb, FB)], ps[:])

                # --- k and v projections, sharing mT chunk loads ---
                for tw in range(NFB):   # t-window of 512
                    chunks = []
                    for k in range(KD):
                        c = rhsp.tile([P, FB], MM, name="mchunk")
                        nc.sync.dma_start(c[:], mTd[ds(k * P, P), ts(tw, FB)])
                        chunks.append(c)
                    for hp in range(2):
                        ps = pps.tile([P, FB], F32)
                        for k in range(KD):
                            nc.tensor.matmul(
                                ps[:], wk_sb[:, ds(k * JC + hp * P, P)],
                                chunks[k][:],
                                start=(k == 0), stop=False)
                        nc.tensor.matmul(
                            ps[:], bk_sb[:, ds(hp * P, P)], ones_row[:],
                            start=False, stop=True)
                        nc.vector.tensor_copy(kTp[hp][:, ts(tw, FB)], ps[:])
                    for s in range(4):
                        t = tw * 4 + s
                        psv = ppv.tile([P, JC], F32)
                        for k in range(KD):
                            nc.tensor.matmul(
                                psv[:], chunks[k][:, ds(s * P, P)],
                                wv_sb[:, ts(k, JC)],
                                start=(k == 0), stop=False)
                        nc.tensor.matmul(
                            psv[:], ones_col[:], bv_sb[:],
                            start=False, stop=True)
                        # evict: v * exp(mb[t]) into strided [t, 4x65] layout
                        dst = v_sb[t].rearrange("p (h c) -> p h c", h=HPC)
                        nc.vector.tensor_scalar_mul(
                            dst[:, :, 0:DH],
                            psv[:].rearrange("p (h c) -> p h c", h=HPC),
                            eb_sb[:, ds(t, 1)])
                        for h in range(HPC):
                            nc.vector.tensor_copy(
                                dst[:, ds(h, 1), ds(DH, 1)], eb_sb[:, ds(t, 1)])

            # ================= phase 2: attention + out-proj =================
            if debug_taps:
                nc.sync.dma_start(taps["t_qTp0"], qTp[0][:])
                nc.sync.dma_start(taps["t_kTp0"], kTp[0][:])
                nc.sync.dma_start(taps["t_v0"], v_sb[0][:])
            wo_sb = [persist.tile([DH, D], MM, name=f"wo{i}") for i in range(HPC)]
            for i in range(HPC):
                nc.sync.dma_start(wo_sb[i][:], wod[ds(i * DH, DH), :])

            with (
                tc.tile_pool(name="expp", bufs=18) as expp,
                tc.tile_pool(name="attnp", bufs=8) as attnp,
                tc.tile_pool(name="rp", bufs=4) as rp,
                tc.tile_pool(name="rbp", bufs=4) as rbp,
                tc.tile_pool(name="osb", bufs=4) as osb,
                tc.tile_pool(name="psl", bufs=3, space="PSUM") as psl,
                tc.tile_pool(name="psx", bufs=2, space="PSUM") as psx,
            ):
                for fb in range(NFB):
                    attn = []   # [64, FB] per head, this f-block
                    for hp in range(2):
                        # logits + exp; one [128, G*FB] psum tile per
                        # (head, t-chunk group), triple-buffered so ScalarE
                        # (the phase-2 bottleneck) never starves
                        exps = [[], []]
                        for g2 in range(2 * NG):
                            h2, g = g2 & 1, g2 >> 1
                            pls = psl.tile([P, G * FB], F32, name="pls")
                            for s in range(G):
                                t = g * G + s
                                nc.tensor.matmul(
                                    pls[:, ts(s, FB)],
                                    kTp[hp][ds(h2 * DH, DH), ts(t, P)],
                                    qTp[hp][ds(h2 * DH, DH), ts(fb, FB)],
                                    start=True, stop=True)
                            e = expp.tile([P, G * FB], MM, name="exps")
                            nc.scalar.activation(
                                e[:], pls[:],
                                mybir.ActivationFunctionType.Exp)
                            exps[h2].append(e)
                            if debug_taps and fb == 0 and hp == 0 \
                                    and g == 0 and h2 == 0:
                                nc.sync.dma_start(taps["t_exp0"], e[:])
                        for h2 in range(2):
                            h = 2 * hp + h2
                            av = psx.tile([P, FB], F32, name="av",
                                          tag="x")[0:VW, :]
                            for g in range(NG):
                                for s in range(G):
                                    t = g * G + s
                                    nc.tensor.matmul(
                                        av[:], v_sb[t][:, ds(h * VS, VW)],
                                        exps[h2][g][:, ts(s, FB)],
                                        start=(t == 0), stop=(t == NTC - 1))
                            # denominator -> reciprocal -> broadcast to rows
                            if debug_taps and fb == 0 and hp == 0 and h2 == 0:
                                avt = rbp.tile([VW, FB], MM, name="avt")
                                nc.vector.tensor_copy(avt[:], av[:])
                                nc.sync.dma_start(taps["t_av0"], avt[:])
                            # custom-DVE ops ignore base_partition on PSUM
                            # inputs: stage the denominator row to SBUF first
                            dn = rp.tile([1, FB], F32, name="dn")
                            nc.vector.tensor_copy(dn[:], av[ds(DH, 1), :])
                            rf = rp.tile([1, FB], F32, name="rf")
                            nc.vector.reciprocal_approx_fast(rf[:], dn[:])
                            rrow = rp.tile([1, FB], MM, name="rrow")
                            nc.vector.tensor_copy(rrow[:], rf[:])
                            if debug_taps and fb == 0 and hp == 0 and h2 == 0:
                                nc.sync.dma_start(taps["t_rf0"], rf[:])
                            rb_ps = psx.tile([P, FB], F32, name="rb_ps",
                                             tag="x")
                            nc.tensor.matmul(rb_ps[:], ones_col[:],
                                             rrow[:], start=True, stop=True)
                            rb = rbp.tile([DH, FB], F32, name="rb")
                            nc.vector.tensor_copy(rb[:], rb_ps[0:DH, :])
                            a = attnp.tile([DH, FB], MM, name="attn")
                            nc.vector.tensor_tensor(
                                a[:], av[0:DH, :], rb[:],
                                op=mybir.AluOpType.mult)
                            if debug_taps and fb == 0 and h == 0:
                                nc.sync.dma_start(taps["t_attn0"], a[:])
                                rbt = rbp.tile([DH, FB], MM, name="rbt")
                                nc.vector.tensor_copy(rbt[:], rb[:])
                                nc.sync.dma_start(taps["t_rb0"], rbt[:])
                            attn.append(a)
                    # ---- output projection for this f-block ----
                    for fc in range(4):
                        for jb in range(2):
                            ops = psx.tile([P, FB], F32, name="ops",
                                           tag="x")
                            for h in range(HPC):
                                nc.tensor.matmul(
                                    ops[:],
                                    attn[h][:, ds(fc * P, P)],
                                    wo_sb[h][:, ts(jb, FB)],
                                    start=(h == 0), stop=(h == HPC - 1))
                            o = osb.tile([P, FB], F32, name="osb")
                            nc.vector.tensor_copy(o[:], ops[:])
                            nc.sync.dma_start(
                                outd[ds(fb * FB + fc * P, P), ts(jb, FB)],
                                o[:])

    nc.compile()
    return nc


_CACHE = {}


def _get_module():
    if "nc" not in _CACHE:
        _CACHE["nc"] = build_kernel()
    return _CACHE["nc"]


def make_in_maps(query, memory, memory_bias, Wq, bq, Wk, bk, Wv, bv, Wo, bo,
                 mm_np=None):
    if mm_np is None:
        import ml_dtypes
        mm_np = ml_dtypes.bfloat16
    query = np.asarray(query, np.float32)
    memory = np.asarray(memory, np.float32)
    memory_bias = np.asarray(memory_bias, np.float32)
    Wq = np.asarray(Wq, np.float32)
    bq = np.asarray(bq, np.float32)
    Wk = np.asarray(Wk, np.float32)
    bk = np.asarray(bk, np.float32)
    Wv = np.asarray(Wv, np.float32)
    bv = np.asarray(bv, np.float32)
    Wo = np.asarray(Wo, np.float32)
    s = np.float32(DH ** -0.5)

    qT = [np.ascontiguousarray(query[b].T).astype(mm_np) for b in range(B)]
    mT = [np.ascontiguousarray(memory[b].T).astype(mm_np) for b in range(B)]
    in_maps = []
    for c in range(NCORE):
        b, g = divmod(c, B * 2)
        J = slice(g * JC, (g + 1) * JC)
        in_maps.append({
            "qT": qT[b],
            "mT": mT[b],
            "wq": (np.ascontiguousarray(Wq[:, J]) * s).astype(mm_np),
            "wk": np.ascontiguousarray(Wk[:, J]).astype(mm_np),
            "wv": np.ascontiguousarray(Wv[:, J]).astype(mm_np),
            "bq": (bq[J] * s).reshape(1, JC).astype(mm_np),
            "bk": bk[J].reshape(1, JC).astype(mm_np),
            "bv": bv[J].reshape(1, JC).astype(mm_np),
            "wo": np.ascontiguousarray(Wo[J, :]).astype(mm_np),
            "mb": memory_bias[b].reshape(NTC, P).copy(),
        })
    return in_maps


def gather_output(results, bo):
    bo = np.asarray(bo, np.float32)
    out = np.empty((B, LQ, D), np.float32)
    for b in range(B):
        acc = results[4 * b]["out"].astype(np.float32)
        for g in range(1, 4):
            acc = acc + results[4 * b + g]["out"]
        out[b] = acc + bo
    return out


def kernel(**inputs):
    nc = _get_module()
    in_maps = make_in_maps(**inputs)
    res = bass_utils.run_bass_kernel_spmd(nc, in_maps,
                                          core_ids=list(range(NCORE)))
    return gather_output(res.results, inputs["bo"])


# revision 19
# speedup vs baseline: 1.5070x; 1.0244x over previous
"""Multi-head attention (B=2, L=2048, D=1024, H=16) on 8 TRN2 NeuronCores.

Sharding: batch (2) x head-group (4 heads each) = 8 shards.
Each core computes q/k/v projections for its 4 heads, attention, and a
partial output projection (its 256 rows of Wo); host sums the 4 partials
per batch and adds bo.

Device dataflow (per core):
  inputs (host-prepped, bf16 except memory_bias):
    qT  [1024, 2048]  = query[b].T          (d on partitions for matmul)
    mT  [1024, 2048]  = memory[b].T
    wq  [1024, 256]   = Wq[:, J] * 0.125    (scale folded)
    wk, wv [1024, 256];  bq*0.125, bk, bv [1, 256]
    wo  [256, 1024]   = Wo[J, :]
    mb  [16, 128]     = memory_bias[b]  (f32)
  phase 1: qT_h [j, f], kT_h [j, t] (head-dim on partitions), v [t, j]
           (natural), biases folded in via K=1 ones matmuls, and
           v scaled by exp(memory_bias[t]) so the softmax bias drops out:
           softmax(s + b) @ v == (exp(s) @ (v * e^b)) / (exp(s) @ e^b)
  phase 2: sT = kT.T@qT (two heads row-tiled to overlap in the PE array)
           -> exp on ScalarE (the phase-2 bottleneck, kept saturated via
           a triple-buffered psum pool) -> AV matmul with an extra
           e^b column producing the softmax denominator row for free ->
           normalize into a [128, f] head-pair tile -> row-paired output
           projection partial [f, j].
"""

import numpy as np

import concourse.bass as bass
import concourse.tile as tile
from concourse import bacc, mybir
from concourse import bass_utils
from concourse.bass import ts, ds

F32 = mybir.dt.float32
F32R = mybir.dt.float32r
BF16 = mybir.dt.bfloat16

B, LQ, LM, D, H = 2, 2048, 2048, 1024, 16
DH = 64
HPC = 4            # heads per core
JC = HPC * DH      # 256 projection cols per core
NCORE = 8
P = 128
FB = 512           # f-block width
NFB = LQ // FB     # 4
NTC = LM // P      # 16 t-chunks
KD = D // P        # 8 contraction chunks for projections
G = 2              # t-chunks per exp group (psum tile [128, G*512])
NG = NTC // G      # 8 groups

VW = DH + 1        # v columns per head incl. denominator column
VS = DH + 2        # v column stride per head (4B alignment in bf16)


def build_kernel(mm_dt=BF16):
    MM = mm_dt
    nc = bacc.Bacc("TRN2", target_bir_lowering=False, debug=False)

    qTd = nc.dram_tensor("qT", [D, LQ], MM, kind="ExternalInput").ap()
    mTd = nc.dram_tensor("mT", [D, LM], MM, kind="ExternalInput").ap()
    wqd = nc.dram_tensor("wq", [D, JC], MM, kind="ExternalInput").ap()
    wkd = nc.dram_tensor("wk", [D, JC], MM, kind="ExternalInput").ap()
    wvd = nc.dram_tensor("wv", [D, JC], MM, kind="ExternalInput").ap()
    bqd = nc.dram_tensor("bq", [1, JC], MM, kind="ExternalInput").ap()
    bkd = nc.dram_tensor("bk", [1, JC], MM, kind="ExternalInput").ap()
    bvd = nc.dram_tensor("bv", [1, JC], MM, kind="ExternalInput").ap()
    wod = nc.dram_tensor("wo", [JC, D], MM, kind="ExternalInput").ap()
    mbd = nc.dram_tensor("mb", [NTC, P], F32, kind="ExternalInput").ap()
    outd = nc.dram_tensor("out", [LQ, D], F32, kind="ExternalOutput").ap()

    with tile.TileContext(nc) as tc:
        with (
            tc.tile_pool(name="persist", bufs=1) as persist,
            tc.tile_pool(name="vpool", bufs=1) as vpool,
            tc.tile_pool(name="consts", bufs=1) as consts,
        ):
            # ---- constants ----
            ones_f = consts.tile([1, FB], F32)
            nc.vector.memset(ones_f[:], 1.0)
            ones_row = consts.tile([1, FB], MM)      # rhs for bias matmuls
            nc.vector.tensor_copy(ones_row[:], ones_f[:])
            ones_col = consts.tile([1, P], MM)       # lhsT for v-bias / R bcast
            nc.vector.tensor_copy(ones_col[:], ones_f[:, 0:P])
            mb_sb = consts.tile([P, NTC], F32)
            nc.sync.dma_start(mb_sb[:], mbd.transpose([1, 0]))
            eb_sb = consts.tile([P, NTC], F32)        # exp(memory_bias), col=tc
            nc.scalar.activation(eb_sb[:], mb_sb[:],
                                 mybir.ActivationFunctionType.Exp)

            # ---- persistent activations ----
            # qT/kT: per head-pair tile [128 (2 heads x 64 dh), L]
            qTp = [persist.tile([P, LQ], MM, name=f"qTp{i}") for i in range(2)]
            kTp = [persist.tile([P, LM], MM, name=f"kTp{i}") for i in range(2)]
            # v: per t-chunk [128 t, 4 heads x (64 v cols + e^b col + pad)]
            v_sb = [vpool.tile([P, HPC * VS], MM, name=f"v{t}")
                    for t in range(NTC)]
            wo_sb = [persist.tile([DH, D], MM, name=f"wo{i}")
                     for i in range(HPC)]
            for i in range(HPC):
                nc.sync.dma_start(wo_sb[i][:], wod[ds(i * DH, DH), :])

            # ================= phase 1: projections =================
            with (
                tc.tile_pool(name="wts", bufs=1) as wts,
                tc.tile_pool(name="rhs", bufs=2 * KD + 2) as rhsp,
                tc.tile_pool(name="pps", bufs=4, space="PSUM") as pps,
                tc.tile_pool(name="ppv", bufs=2, space="PSUM") as ppv,
            ):
                wq_sb = wts.tile([P, KD * JC], MM, name="wq")
                wk_sb = wts.tile([P, KD * JC], MM, name="wk")
                wv_sb = wts.tile([P, KD * JC], MM, name="wv")
                for w_sb, wd in ((wq_sb, wqd), (wk_sb, wkd), (wv_sb, wvd)):
                    for k in range(KD):
                        nc.sync.dma_start(w_sb[:, ts(k, JC)], wd[ds(k * P, P), :])
                bq_sb = wts.tile([1, JC], MM, name="bq")
                bk_sb = wts.tile([1, JC], MM, name="bk")
                bv_sb = wts.tile([1, JC], MM, name="bv")
                nc.sync.dma_start(bq_sb[:], bqd[:])
                nc.sync.dma_start(bk_sb[:], bkd[:])
                nc.sync.dma_start(bv_sb[:], bvd[:])

                # interleave k/v t-windows with q f-blocks so phase 2's
                # first logits groups become runnable as early as possible
                for rnd in range(NFB):
                    # --- k + v for t-window rnd ---
                    chunks = []
                    for k in range(KD):
                        c = rhsp.tile([P, FB], MM, name="mchunk")
                        nc.sync.dma_start(c[:], mTd[ds(k * P, P), ts(rnd, FB)])
                        chunks.append(c)
                    for hp in range(2):
                        ps = pps.tile([P, FB], F32)
                        for k in range(KD):
                            nc.tensor.matmul(
                                ps[:], wk_sb[:, ds(k * JC + hp * P, P)],
                                chunks[k][:],
                                start=(k == 0), stop=False)
                        nc.tensor.matmul(
                            ps[:], bk_sb[:, ds(hp * P, P)], ones_row[:],
                            start=False, stop=True)
                        nc.vector.tensor_copy(kTp[hp][:, ts(rnd, FB)], ps[:])
                    for s in range(4):
                        t = rnd * 4 + s
                        psv = ppv.tile([P, JC], F32)
                        for k in range(KD):
                            nc.tensor.matmul(
                                psv[:], chunks[k][:, ds(s * P, P)],
                                wv_sb[:, ts(k, JC)],
                                start=(k == 0), stop=False)
                        nc.tensor.matmul(
                            psv[:], ones_col[:], bv_sb[:],
                            start=False, stop=True)
                        # evict: v * exp(mb[t]) into strided [t, 4xVS] layout
                        dst = v_sb[t].rearrange("p (h c) -> p h c", h=HPC)
                        nc.vector.tensor_scalar_mul(
                            dst[:, :, 0:DH],
                            psv[:].rearrange("p (h c) -> p h c", h=HPC),
                            eb_sb[:, ds(t, 1)])
                        for h in range(HPC):
                            nc.vector.tensor_copy(
                                dst[:, ds(h, 1), ds(DH, 1)], eb_sb[:, ds(t, 1)])
                    # --- q for f-block rnd ---
                    chunks = []
                    for k in range(KD):
                        c = rhsp.tile([P, FB], MM, name="qchunk")
                        nc.sync.dma_start(c[:], qTd[ds(k * P, P), ts(rnd, FB)])
                        chunks.append(c)
                    for hp in range(2):
                        ps = pps.tile([P, FB], F32)
                        for k in range(KD):
                            nc.tensor.matmul(
                                ps[:], wq_sb[:, ds(k * JC + hp * P, P)],
                                chunks[k][:],
                                start=(k == 0), stop=False)
                        nc.tensor.matmul(
                            ps[:], bq_sb[:, ds(hp * P, P)], ones_row[:],
                            start=False, stop=True)
                        nc.vector.tensor_copy(qTp[hp][:, ts(rnd, FB)], ps[:])

            # ================= phase 2: attention + out-proj =================
            with (
                tc.tile_pool(name="expp", bufs=18) as expp,
                tc.tile_pool(name="attnp", bufs=8) as attnp,
                tc.tile_pool(name="rp", bufs=6) as rp,
                tc.tile_pool(name="rbp", bufs=4) as rbp,
                tc.tile_pool(name="osb", bufs=4) as osb,
                tc.tile_pool(name="psl", bufs=3, space="PSUM") as psl,
                tc.tile_pool(name="psx", bufs=2, space="PSUM") as psx,
            ):
                for fb in range(NFB):
                    attn = []   # [128, FB] head-pair tile per hp, this f-block
                    for hp in range(2):
                        # logits + exp: per group 2 psum tiles (one per head);
                        # the two heads' K=64 matmuls are interleaved so they
                        # row-tile into strips 0-63 / 64-127 concurrently
                        exps = [[], []]
                        for g in range(NG):
                            pls = [psl.tile([P, G * FB], F32, name="pls")
                                   for _ in range(2)]
                            for s in range(G):
                                t = g * G + s
                                for h2 in range(2):
                                    nc.tensor.matmul(
                                        pls[h2][:, ts(s, FB)],
                                        kTp[hp][ds(h2 * DH, DH), ts(t, P)],
                                        qTp[hp][ds(h2 * DH, DH), ts(fb, FB)],
                                        start=True, stop=True)
                            for h2 in range(2):
                                e = expp.tile([P, G * FB], MM, name="exps")
                                nc.scalar.activation(
                                    e[:], pls[h2][:],
                                    mybir.ActivationFunctionType.Exp)
                                exps[h2].append(e)
                        for h2 in range(2):
                            h = 2 * hp + h2
                            av = psx.tile([P, FB], F32, name="av",
                                          tag="x")[0:VW, :]
                            for g in range(NG):
                                for s in range(G):
                                    t = g * G + s
                                    nc.tensor.matmul(
                                        av[:], v_sb[t][:, ds(h * VS, VW)],
                                        exps[h2][g][:, ts(s, FB)],
                                        start=(t == 0), stop=(t == NTC - 1))
                            # denominator -> reciprocal -> PE row-broadcast
                            # (custom-DVE ops ignore base_partition on PSUM
                            # inputs: stage the denom row to SBUF first)
                            dn = rp.tile([1, FB], F32, name="dn")
                            nc.vector.tensor_copy(dn[:], av[ds(DH, 1), :])
                            rf = rp.tile([1, FB], F32, name="rf")
                            nc.vector.reciprocal_approx_fast(rf[:], dn[:])
                            rrow = rp.tile([1, FB], MM, name="rrow")
                            nc.vector.tensor_copy(rrow[:], rf[:])
                            rb_ps = psx.tile([P, FB], F32, name="rb_ps",
                                             tag="x")
                            nc.tensor.matmul(rb_ps[:], ones_col[:],
                                             rrow[:], start=True, stop=True)
                            rb = rbp.tile([DH, FB], F32, name="rb")
                            nc.vector.tensor_copy(rb[:], rb_ps[0:DH, :])
                            a = attnp.tile([DH, FB], MM, name="attn")
                            nc.vector.tensor_tensor(
                                a[:], av[0:DH, :], rb[:],
                                op=mybir.AluOpType.mult)
                            attn.append(a)
                    # ---- output projection for this f-block ----
                    for fc in range(4):
                        for jb in range(2):
                            ops = psx.tile([P, FB], F32, name="ops", tag="x")
                            for h in range(HPC):
                                nc.tensor.matmul(
                                    ops[:],
                                    attn[h][:, ds(fc * P, P)],
                                    wo_sb[h][:, ts(jb, FB)],
                                    start=(h == 0), stop=(h == HPC - 1))
                            o = osb.tile([P, FB], F32, name="osb")
                            nc.vector.tensor_copy(o[:], ops[:])
                            nc.sync.dma_start(
                                outd[ds(fb * FB + fc * P, P), ts(jb, FB)],
                                o[:])

    nc.compile()
    return nc


_CACHE = {}


def _get_module():
    if "nc" not in _CACHE:
        _CACHE["nc"] = build_kernel()
    return _CACHE["nc"]


def make_in_maps(query, memory, memory_bias, Wq, bq, Wk, bk, Wv, bv, Wo, bo,
                 mm_np=None):
    if mm_np is None:
        import ml_dtypes
        mm_np = ml_dtypes.bfloat16
    query = np.asarray(query, np.float32)
    memory = np.asarray(memory, np.float32)
    memory_bias = np.asarray(memory_bias, np.float32)
    Wq = np.asarray(Wq, np.float32)
    bq = np.asarray(bq, np.float32)
    Wk = np.asarray(Wk, np.float32)
    bk = np.asarray(bk, np.float32)
    Wv = np.asarray(Wv, np.float32)
    bv = np.asarray(bv, np.float32)
    Wo = np.asarray(Wo, np.float32)
    s = np.float32(DH ** -0.5)

    qT = [np.ascontiguousarray(query[b].T).astype(mm_np) for b in range(B)]
    mT = [np.ascontiguousarray(memory[b].T).astype(mm_np) for b in range(B)]
    in_maps = []
    for c in range(NCORE):
        b, g = divmod(c, 4)
        J = slice(g * JC, (g + 1) * JC)
        in_maps.append({
            "qT": qT[b],
            "mT": mT[b],
            "wq": (np.ascontiguousarray(Wq[:, J]) * s).astype(mm_np),
            "wk": np.ascontiguousarray(Wk[:, J]).astype(mm_np),
            "wv": np.ascontiguousarray(Wv[:, J]).astype(mm_np),
            "bq": (bq[J] * s).reshape(1, JC).astype(mm_np),
            "bk": bk[J].reshape(1, JC).astype(mm_np),
            "bv": bv[J].reshape(1, JC).astype(mm_np),
            "wo": np.ascontiguousarray(Wo[J, :]).astype(mm_np),
            "mb": memory_bias[b].reshape(NTC, P).copy(),
        })
    return in_maps


def gather_output(results, bo):
    bo = np.asarray(bo, np.float32)
    out = np.empty((B, LQ, D), np.float32)
    for b in range(B):
        acc = results[4 * b]["out"].astype(np.float32)
        for g in range(1, 4):
            acc = acc + results[4 * b + g]["out"]
        out[b] = acc + bo
    return out


def kernel(**inputs):
    nc = _get_module()
    in_maps = make_in_maps(**inputs)
    res = bass_utils.run_bass_kernel_spmd(nc, in_maps,
                                          core_ids=list(range(NCORE)))
    return gather_output(res.results, inputs["bo"])
